# revision 1
# baseline (speedup 1.0000x reference)
"""Expert-parallel MoE (top-2 of 8 experts, SwiGLU FFN) for 8 Trainium2 cores.

Strategy (matches the expert-parallel sharding hint):
  - Host computes the small gate (logits -> top-2 -> softmax) in float64
    numpy, then dispatches ("all-to-all" on host) tokens to experts.
  - Core e holds expert e's weights and runs a dense SwiGLU FFN over the
    tokens routed to expert e (padded to a common capacity C so all 8
    cores run the same SPMD program).
  - The device kernel works entirely in "feature-major" layout (features
    on partitions, tokens on the free axis) so the h = silu(x@Wg)*(x@Wu)
    intermediate feeds the down-projection without any transpose.
  - Host applies the routing weights and scatter-adds the per-expert
    outputs back into the full [B,T,D] output.

DTYPE selects the matmul path (both measured on HW):
  - "f32r" (default): fp32 bits on the relaxed-precision PE path,
    1 cycle/row + ~40 cycle/matmul weight-load bubble. Max-core HW time
    ~561 us, rel err vs f64 reference 2.6e-4.
  - "bf16": inputs rounded to bfloat16, 1 cycle/row, LDWEIGHTS hidden by
    fast-weight-load. ~508 us but rel err 4.1e-3 — kept as a fallback.
"""

import numpy as np

DIM = 1024
HID = 2816
E = 8
TOPK = 2
P = 128
KD = DIM // P   # 8 k-subtiles (contraction of x@W)
HT = HID // P   # 22 h-subtiles
DT = DIM // P   # 8 d-subtiles (output features)

DTYPE = "f32r"          # "bf16" | "f32r"
CHUNK_MAX = 512         # PSUM bank limit (512 fp32 accumulators)
CHUNK_MIN = 384         # keep weight-DMA per chunk sustainable

_KERNEL_CACHE = {}
LAST_RESULTS = None  # BassKernelResults of the most recent run (for profiling)


def _align():
    # f32r matmuls reject odd moving free dims (walrus ISA check);
    # bf16 takes any size.
    return 1 if DTYPE == "bf16" else 2


def _capacity(max_cnt):
    # chunks produced by _build_chunks are all >= CHUNK_MIN >= 256, so
    # both the bf16 and f32r matmul fast paths allow any capacity.
    a = _align()
    return max(CHUNK_MIN, ((max_cnt + a - 1) // a) * a)


def _build_chunks(C):
    """Split C into aligned chunks in [CHUNK_MIN, CHUNK_MAX], largest
    LAST: a trailing single-chunk weight group streams 1MB of wg/wu per
    h-tile, and only a full-width chunk keeps that under the ~360GB/s
    per-core HBM roofline."""
    count = (C + CHUNK_MAX - 1) // CHUNK_MAX
    sizes = []
    rem = C
    for i in range(count, 0, -1):
        if i == 1:
            s = rem
        else:
            s = min(CHUNK_MAX, rem - CHUNK_MIN * (i - 1))
        sizes.append(s)
        rem -= s
    sizes.reverse()
    assert all(CHUNK_MIN <= s <= CHUNK_MAX and s % _align() == 0 for s in sizes)
    chunks = []
    off = 0
    for s in sizes:
        chunks.append((off, s))
        off += s
    return chunks


def _build_groups(chunks, group_size):
    return [chunks[i : i + group_size] for i in range(0, len(chunks), group_size)]


def _build_moe_ffn(C):
    """Build the per-core Bass program: y^T = SwiGLU FFN of x^T, both
    feature-major, tokens padded to capacity C."""
    import concourse.bass as bass  # noqa: F401
    import concourse.mybir as mybir
    from concourse import bacc, tile

    f32 = mybir.dt.float32
    dt_in = mybir.dt.bfloat16 if DTYPE == "bf16" else mybir.dt.float32r
    SiLU = mybir.ActivationFunctionType.Silu

    nc = bacc.Bacc("TRN2", target_bir_lowering=False, debug=False)

    xt = nc.dram_tensor("xt", [P, KD, C], dt_in, kind="ExternalInput")
    wgt = nc.dram_tensor("wgt", [HT, P, KD, P], dt_in, kind="ExternalInput")
    wut = nc.dram_tensor("wut", [HT, P, KD, P], dt_in, kind="ExternalInput")
    wdt = nc.dram_tensor("wdt", [DT, P, HT, P], dt_in, kind="ExternalInput")
    yt = nc.dram_tensor("yt", [DT, P, C], f32, kind="ExternalOutput")

    # bf16 halves weight DMA, so single-chunk groups sustain; f32r needs
    # two chunks per weight pass to stay under the HBM roofline.
    group_size = 1 if DTYPE == "bf16" else 2
    groups = _build_groups(_build_chunks(C), group_size)

    with tile.TileContext(nc) as tc:
        with (
            tc.tile_pool(name="xp", bufs=1) as xp,
            tc.tile_pool(name="wp", bufs=3) as wp,
            tc.tile_pool(name="hp", bufs=2 if group_size == 1 else 1) as hp,
            tc.tile_pool(name="op", bufs=3) as op,
            tc.tile_pool(name="ps", bufs=2, space="PSUM") as ps,
        ):
            HH = KD // 2  # wg/wu tiles split in halves for earlier start
            for group in groups:
                g_off = group[0][0]
                g_size = sum(c[1] for c in group)

                # h = silu(x @ Wg) * (x @ Wu), feature-major [HID, g_size]
                h_sb = hp.tile([P, HT, g_size], dt_in, tag="h")

                # ht=0 weights first so the opening matmuls wait on
                # ~0.75MB, not the whole group's activations
                w_cache = {}

                def load_w(ht):
                    # both wg halves before wu: the opening matmuls of
                    # each h-tile consume wg only
                    wg_sb, wu_sb = [], []
                    for hh in range(2):
                        w1 = wp.tile([P, HH, P], dt_in, tag=f"wg{hh}",
                                     name=f"wg{hh}")
                        nc.sync.dma_start(
                            w1[:], wgt[ht, :, hh * HH : (hh + 1) * HH])
                        wg_sb.append(w1)
                    for hh in range(2):
                        w2 = wp.tile([P, HH, P], dt_in, tag=f"wu{hh}",
                                     name=f"wu{hh}")
                        nc.sync.dma_start(
                            w2[:], wut[ht, :, hh * HH : (hh + 1) * HH])
                        wu_sb.append(w2)
                    w_cache[ht] = (wg_sb, wu_sb)

                load_w(0)

                # first k-slice of x as its own tile so the opening
                # matmuls don't wait for the whole chunk's activations
                x_sb = []
                for gi, (off, csize) in enumerate(group):
                    x0 = xp.tile([P, csize], dt_in, tag=f"x{gi}k0",
                                 bufs=2 if gi == 0 else 1, name=f"x{gi}k0")
                    nc.sync.dma_start(x0[:], xt[:, 0, off : off + csize])
                    xr = xp.tile([P, KD - 1, csize], dt_in, tag=f"x{gi}r",
                                 bufs=2 if gi == 0 else 1, name=f"x{gi}r")
                    nc.sync.dma_start(xr[:], xt[:, 1:, off : off + csize])
                    x_sb.append([x0] + [xr[:, kt] for kt in range(KD - 1)])

                for ht in range(HT):
                    if ht not in w_cache:
                        load_w(ht)
                    wg_sb, wu_sb = w_cache.pop(ht)

                    for gi, (off, csize) in enumerate(group):
                        pg = ps.tile([P, csize], f32, tag="pg", bufs=3)
                        pu = ps.tile([P, csize], f32, tag="pu", bufs=3)
                        for kt in range(KD):
                            nc.tensor.matmul(
                                pg,
                                wg_sb[kt // HH][:, kt % HH],
                                x_sb[gi][kt],
                                start=(kt == 0),
                                stop=(kt == KD - 1),
                            )
                        for kt in range(KD):
                            nc.tensor.matmul(
                                pu,
                                wu_sb[kt // HH][:, kt % HH],
                                x_sb[gi][kt],
                                start=(kt == 0),
                                stop=(kt == KD - 1),
                            )
                        sl = op.tile([P, csize], f32, tag="silu")
                        nc.scalar.activation(sl[:], pg, SiLU)
                        lo = off - g_off
                        nc.vector.tensor_mul(
                            h_sb[:, ht, lo : lo + csize], sl[:], pu
                        )

                # y = h @ Wd, feature-major [DIM, g_size]
                for dt in range(DT):
                    wd_sb = wp.tile([P, HT, P], dt_in, tag="wd")
                    nc.sync.dma_start(wd_sb[:], wdt[dt])
                    for gi, (off, csize) in enumerate(group):
                        py = ps.tile([P, csize], f32, tag="py")
                        lo = off - g_off
                        for ht in range(HT):
                            nc.tensor.matmul(
                                py,
                                wd_sb[:, ht],
                                h_sb[:, ht, lo : lo + csize],
                                start=(ht == 0),
                                stop=(ht == HT - 1),
                            )
                        o_sb = op.tile([P, csize], f32, tag="o")
                        nc.vector.tensor_copy(o_sb[:], py)
                        nc.sync.dma_start(yt[dt, :, off : off + csize], o_sb[:])

    nc.finalize()
    return nc


def _get_kernel(C):
    if C not in _KERNEL_CACHE:
        _KERNEL_CACHE[C] = _build_moe_ffn(C)
    return _KERNEL_CACHE[C]


def _np_dtype():
    if DTYPE == "bf16":
        import ml_dtypes

        return np.dtype(ml_dtypes.bfloat16)
    return np.dtype(np.float32)


def _route(xf, W_gate):
    """Replicate reference routing: top-2 by logit, softmax weights.

    float64 logits: the top-k decision boundary gap is >> f32 rounding
    noise, so this matches the f32 jax reference's selection."""
    logits = xf.astype(np.float64) @ W_gate.astype(np.float64)  # [N, E]
    order = np.argsort(-logits, axis=1, kind="stable")[:, :TOPK]  # [N, 2]
    top = np.take_along_axis(logits, order, axis=1)
    top = top - top.max(axis=1, keepdims=True)
    ew = np.exp(top)
    w = (ew / ew.sum(axis=1, keepdims=True)).astype(np.float32)  # [N, 2]
    return order, w


def kernel(x, W_gate, Wg, Wu, Wd):
    from concourse.bass_utils import run_bass_kernel_spmd

    x = np.ascontiguousarray(np.asarray(x, dtype=np.float32))
    W_gate = np.asarray(W_gate, dtype=np.float32)
    Wg = np.asarray(Wg, dtype=np.float32)
    Wu = np.asarray(Wu, dtype=np.float32)
    Wd = np.asarray(Wd, dtype=np.float32)

    B, T, D = x.shape
    xf = x.reshape(-1, D)
    N = xf.shape[0]

    order, w = _route(xf, W_gate)

    ids = []  # per-expert token indices
    wts = []  # per-expert combine weights
    for e in range(E):
        sel = np.nonzero(order == e)
        ids.append(sel[0])
        wts.append(w[sel[0], sel[1]])

    max_cnt = max(len(i) for i in ids)
    C = _capacity(max_cnt)

    nc = _get_kernel(C)
    ndt = _np_dtype()

    in_maps = []
    for e in range(E):
        cnt = len(ids[e])
        xe = np.zeros((C, DIM), dtype=np.float32)
        xe[:cnt] = xf[ids[e]]
        x_t = np.ascontiguousarray(
            xe.T.reshape(KD, P, C).transpose(1, 0, 2).astype(ndt, copy=False)
        )
        wg_t = np.ascontiguousarray(
            Wg[e].reshape(KD, P, HT, P).transpose(2, 1, 0, 3).astype(ndt, copy=False)
        )
        wu_t = np.ascontiguousarray(
            Wu[e].reshape(KD, P, HT, P).transpose(2, 1, 0, 3).astype(ndt, copy=False)
        )
        wd_t = np.ascontiguousarray(
            Wd[e].reshape(HT, P, DT, P).transpose(2, 1, 0, 3).astype(ndt, copy=False)
        )
        in_maps.append({"xt": x_t, "wgt": wg_t, "wut": wu_t, "wdt": wd_t})

    res = run_bass_kernel_spmd(nc, in_maps, core_ids=list(range(E)))
    global LAST_RESULTS
    LAST_RESULTS = res

    out = np.zeros((N, D), dtype=np.float32)
    for e in range(E):
        cnt = len(ids[e])
        y_e = res.results[e]["yt"].reshape(DIM, C)[:, :cnt].T  # [cnt, D]
        out[ids[e]] += wts[e][:, None] * y_e
    return out.reshape(B, T, D)



# revision 2
# speedup vs baseline: 1.1135x; 1.1135x over previous
"""Expert-parallel MoE (top-2 of 8 experts, SwiGLU FFN) for 8 Trainium2 cores.

Strategy (expert-parallel per the sharding hint, plus load balancing):
  - Host computes the small gate (logits -> top-2 -> softmax) in float64
    numpy, then dispatches tokens to experts.
  - Device work is the 3-matmul SwiGLU FFN in bf16 (1 PE cycle/row, FWL
    hides LDWEIGHTS; rel err ~4e-3 vs the 2e-2 gate). fp8-DoubleRow was
    measured unusable: e4m3 on any one matmul already gives >3.7e-2.
  - Load balancing: per-expert token counts are imbalanced (max 2151 vs
    mean 2048).  Instead of padding every core to the max count, each
    core runs THREE fixed-size slots (a, b, c); the biggest expert's
    tokens span two a-slots (on different cores), the smallest expert
    donates capacity by using two b-slots, and everyone gets one
    c-slot.  This brings per-core capacity S = a+b+c down to
    max(d2, ceil((d1+d8)/2)) ~ 2068 instead of 2152.  Each slot streams
    its own expert's weights (3 x 17.3MB bf16 per core, well under the
    HBM roofline vs ~455us of compute).
  - The device kernel works in "feature-major" layout (features on
    partitions, tokens on the free axis) so h = silu(x@Wg)*(x@Wu)
    feeds the down-projection without any transpose.
  - Host applies routing weights and scatter-adds per-slot outputs
    back into the full [B,T,D] output.
"""

import numpy as np

DIM = 1024
HID = 2816
E = 8
TOPK = 2
P = 128
KD = DIM // P   # 8 k-subtiles (contraction of x@W)
HT = HID // P   # 22 h-subtiles
DT = DIM // P   # 8 d-subtiles (output features)

CHUNK_MAX = 512         # PSUM bank limit (512 fp32 accumulators)

_KERNEL_CACHE = {}
LAST_RESULTS = None  # BassKernelResults of the most recent run (for profiling)


def _ceil_even(v):
    return ((int(v) + 1) // 2) * 2


def _split_chunks(size):
    """Split a slot into even chunks <= CHUNK_MAX (one PSUM bank each),
    near-equal so every chunk's matmuls stay long enough to hide
    LDWEIGHTS."""
    n = -(-size // CHUNK_MAX)
    sizes = []
    rem = size
    for i in range(n, 0, -1):
        s = rem if i == 1 else min(CHUNK_MAX, ((rem // i + 1) // 2) * 2)
        sizes.append(s)
        rem -= s
    assert rem == 0 and all(0 < s <= CHUNK_MAX and s % 2 == 0 for s in sizes)
    chunks = []
    off = 0
    for s in sizes:
        chunks.append((off, s))
        off += s
    return chunks


def _plan(counts):
    """Choose per-core slot sizes + (expert, lo, hi) piece assignment.

    Slot types (a, b, c), 8 copies of each (one per core):
      biggest expert  -> {a, a, c}   (its surplus spans two cores' a-slots)
      smallest expert -> {b, b, c}   (donates capacity: b < a)
      middle experts  -> {a, b, c}
    Coverage: 2a+c >= d_max, 2b+c >= d_min, a+b+c >= d_2nd, so
    S = a+b+c = max(d_2nd, ceil((d_max+d_min)/2)) is achievable.
    Returns (sizes, assign) with sizes desc-sorted and
    assign[core] = [(expert, lo, hi), ...] parallel to sizes.
    """
    cnts = [int(x) for x in counts]
    order = sorted(range(E), key=lambda e: -cnts[e])
    d = [cnts[e] for e in order]
    d0, d1, dmin = d[0], d[1], d[-1]

    smin = _ceil_even(max(d1, (d0 + dmin + 1) // 2, 6))
    best = None
    for c in range(384, 1153, 32):
        if c >= smin - 4:
            break
        S = smin
        plan_ab = None
        while S <= _ceil_even(d0):
            a = _ceil_even(-(-max(d0 - c, 2) // 2))
            b = S - a - c
            if b >= 2 and 2 * b + c >= dmin:
                plan_ab = (a, b)
                break
            S += 2
        if plan_ab is None:
            continue
        a, b = plan_ab
        minchunk = min(
            min(cs for _, cs in _split_chunks(s)) for s in (a, b, c)
        )
        key = (S, -minchunk)
        if best is None or key < best[0]:
            best = (key, (a, b, c))

    if best is None:
        # fallback: uniform single slot per core (always correct)
        C = _ceil_even(max(max(cnts), 256))
        sizes = [C]
        assign = [[(order[i], 0, cnts[order[i]])] for i in range(E)]
        return sizes, assign

    a, b, c = best[1]

    def pieces(n, caps):
        out = []
        lo = 0
        for cap in caps:
            hi = min(n, lo + cap)
            out.append((lo, hi))
            lo = hi
        assert lo == n
        return out

    a_pieces, b_pieces, c_pieces = [], [], []
    for rank, e in enumerate(order):
        n = cnts[e]
        if rank == 0:
            pa1, pa2, pc = pieces(n, [a, a, c])
            a_pieces += [(e,) + pa1, (e,) + pa2]
            c_pieces.append((e,) + pc)
        elif rank == E - 1:
            pb1, pb2, pc = pieces(n, [b, b, c])
            b_pieces += [(e,) + pb1, (e,) + pb2]
            c_pieces.append((e,) + pc)
        else:
            pa, pb, pc = pieces(n, [a, b, c])
            a_pieces.append((e,) + pa)
            b_pieces.append((e,) + pb)
            c_pieces.append((e,) + pc)
    assert len(a_pieces) == E and len(b_pieces) == E and len(c_pieces) == E

    typed = sorted(
        [(a, a_pieces), (b, b_pieces), (c, c_pieces)], key=lambda t: -t[0]
    )
    sizes = [t[0] for t in typed]
    assign = [[typed[s][1][i] for s in range(len(typed))] for i in range(E)]
    return sizes, assign


def _build_moe_ffn(slot_sizes):
    """Per-core Bass program: for each slot, y^T = SwiGLU FFN of x^T with
    that slot's own expert weights; feature-major, bf16 matmuls."""
    import concourse.bass as bass  # noqa: F401
    import concourse.mybir as mybir
    from concourse import bacc, tile

    f32 = mybir.dt.float32
    dt_in = mybir.dt.bfloat16
    SiLU = mybir.ActivationFunctionType.Silu

    nc = bacc.Bacc("TRN2", target_bir_lowering=False, debug=False)

    xt, wgt, wut, wdt, yt = [], [], [], [], []
    for s, size in enumerate(slot_sizes):
        xt.append(nc.dram_tensor(f"xt{s}", [P, KD, size], dt_in,
                                 kind="ExternalInput"))
        wgt.append(nc.dram_tensor(f"wgt{s}", [HT, P, KD, P], dt_in,
                                  kind="ExternalInput"))
        wut.append(nc.dram_tensor(f"wut{s}", [HT, P, KD, P], dt_in,
                                  kind="ExternalInput"))
        wdt.append(nc.dram_tensor(f"wdt{s}", [DT, P, HT, P], dt_in,
                                  kind="ExternalInput"))
        yt.append(nc.dram_tensor(f"yt{s}", [DT, P, size], f32,
                                 kind="ExternalOutput"))

    with tile.TileContext(nc) as tc:
        with (
            tc.tile_pool(name="xp", bufs=1) as xp,
            tc.tile_pool(name="wp", bufs=3) as wp,
            tc.tile_pool(name="hp", bufs=2) as hp,
            tc.tile_pool(name="op", bufs=3) as op,
            tc.tile_pool(name="ps", bufs=2, space="PSUM") as ps,
        ):
            HH = KD // 2  # wg/wu tiles split in halves for earlier start
            for s, size in enumerate(slot_sizes):
                group = _split_chunks(size)

                # h = silu(x @ Wg) * (x @ Wu), feature-major [HID, size]
                h_sb = hp.tile([P, HT, size], dt_in, tag="h")

                # ht=0 weights first so the opening matmuls wait on
                # ~0.75MB, not the whole slot's activations
                w_cache = {}

                def load_w(ht, s=s):
                    # both wg halves before wu: the opening matmuls of
                    # each h-tile consume wg only
                    wg_sb, wu_sb = [], []
                    for hh in range(2):
                        w1 = wp.tile([P, HH, P], dt_in, tag=f"wg{hh}",
                                     name=f"wg{hh}")
                        nc.sync.dma_start(
                            w1[:], wgt[s][ht, :, hh * HH : (hh + 1) * HH])
                        wg_sb.append(w1)
                    for hh in range(2):
                        w2 = wp.tile([P, HH, P], dt_in, tag=f"wu{hh}",
                                     name=f"wu{hh}")
                        nc.sync.dma_start(
                            w2[:], wut[s][ht, :, hh * HH : (hh + 1) * HH])
                        wu_sb.append(w2)
                    return (wg_sb, wu_sb)

                w_cache[0] = load_w(0)

                # first k-slice of x as its own tile so the opening
                # matmuls don't wait for the whole chunk's activations
                x_sb = []
                for gi, (off, csize) in enumerate(group):
                    x0 = xp.tile([P, csize], dt_in, tag=f"x{gi}k0",
                                 bufs=2 if gi == 0 else 1, name=f"x{gi}k0")
                    nc.sync.dma_start(x0[:], xt[s][:, 0, off : off + csize])
                    xr = xp.tile([P, KD - 1, csize], dt_in, tag=f"x{gi}r",
                                 bufs=2 if gi == 0 else 1, name=f"x{gi}r")
                    nc.sync.dma_start(xr[:], xt[s][:, 1:, off : off + csize])
                    x_sb.append([x0] + [xr[:, kt] for kt in range(KD - 1)])

                for ht in range(HT):
                    if ht not in w_cache:
                        w_cache[ht] = load_w(ht)
                    wg_sb, wu_sb = w_cache.pop(ht)

                    for gi, (off, csize) in enumerate(group):
                        pg = ps.tile([P, csize], f32, tag="pg", bufs=3)
                        pu = ps.tile([P, csize], f32, tag="pu", bufs=3)
                        for kt in range(KD):
                            nc.tensor.matmul(
                                pg,
                                wg_sb[kt // HH][:, kt % HH],
                                x_sb[gi][kt],
                                start=(kt == 0),
                                stop=(kt == KD - 1),
                            )
                        for kt in range(KD):
                            nc.tensor.matmul(
                                pu,
                                wu_sb[kt // HH][:, kt % HH],
                                x_sb[gi][kt],
                                start=(kt == 0),
                                stop=(kt == KD - 1),
                            )
                        sl = op.tile([P, csize], f32, tag="silu")
                        nc.scalar.activation(sl[:], pg, SiLU)
                        nc.vector.tensor_mul(
                            h_sb[:, ht, off : off + csize], sl[:], pu
                        )

                # y = h @ Wd, feature-major [DIM, size]
                for dt in range(DT):
                    wd_sb = wp.tile([P, HT, P], dt_in, tag="wd")
                    nc.sync.dma_start(wd_sb[:], wdt[s][dt])
                    for gi, (off, csize) in enumerate(group):
                        py = ps.tile([P, csize], f32, tag="py")
                        for ht in range(HT):
                            nc.tensor.matmul(
                                py,
                                wd_sb[:, ht],
                                h_sb[:, ht, off : off + csize],
                                start=(ht == 0),
                                stop=(ht == HT - 1),
                            )
                        o_sb = op.tile([P, csize], f32, tag="o")
                        nc.vector.tensor_copy(o_sb[:], py)
                        nc.sync.dma_start(
                            yt[s][dt, :, off : off + csize], o_sb[:])

    nc.finalize()
    return nc


def _get_kernel(slot_sizes):
    key = tuple(slot_sizes)
    if key not in _KERNEL_CACHE:
        _KERNEL_CACHE[key] = _build_moe_ffn(list(slot_sizes))
    return _KERNEL_CACHE[key]


def _np_bf16():
    import ml_dtypes

    return np.dtype(ml_dtypes.bfloat16)


def _route(xf, W_gate):
    """Replicate reference routing: top-2 by logit, softmax weights.

    float64 logits: the top-k decision boundary gap is >> f32 rounding
    noise, so this matches the f32 jax reference's selection."""
    logits = xf.astype(np.float64) @ W_gate.astype(np.float64)  # [N, E]
    order = np.argsort(-logits, axis=1, kind="stable")[:, :TOPK]  # [N, 2]
    top = np.take_along_axis(logits, order, axis=1)
    top = top - top.max(axis=1, keepdims=True)
    ew = np.exp(top)
    w = (ew / ew.sum(axis=1, keepdims=True)).astype(np.float32)  # [N, 2]
    return order, w


def kernel(x, W_gate, Wg, Wu, Wd):
    from concourse.bass_utils import run_bass_kernel_spmd

    x = np.ascontiguousarray(np.asarray(x, dtype=np.float32))
    W_gate = np.asarray(W_gate, dtype=np.float32)
    Wg = np.asarray(Wg, dtype=np.float32)
    Wu = np.asarray(Wu, dtype=np.float32)
    Wd = np.asarray(Wd, dtype=np.float32)

    B, T, D = x.shape
    xf = x.reshape(-1, D)
    N = xf.shape[0]

    order, w = _route(xf, W_gate)

    ids = []  # per-expert token indices
    wts = []  # per-expert combine weights
    for e in range(E):
        sel = np.nonzero(order == e)
        ids.append(sel[0])
        wts.append(w[sel[0], sel[1]])

    sizes, assign = _plan([len(i) for i in ids])
    nc = _get_kernel(sizes)
    ndt = _np_bf16()

    # transpose + bf16-cast each expert's weights once
    wcache = {}

    def get_w(e):
        if e not in wcache:
            wg_t = np.ascontiguousarray(
                Wg[e].reshape(KD, P, HT, P).transpose(2, 1, 0, 3)
                .astype(ndt, copy=False))
            wu_t = np.ascontiguousarray(
                Wu[e].reshape(KD, P, HT, P).transpose(2, 1, 0, 3)
                .astype(ndt, copy=False))
            wd_t = np.ascontiguousarray(
                Wd[e].reshape(HT, P, DT, P).transpose(2, 1, 0, 3)
                .astype(ndt, copy=False))
            wcache[e] = (wg_t, wu_t, wd_t)
        return wcache[e]

    in_maps = []
    for core in range(E):
        im = {}
        for s, size in enumerate(sizes):
            e, lo, hi = assign[core][s]
            cnt = hi - lo
            xe = np.zeros((size, DIM), dtype=np.float32)
            if cnt:
                xe[:cnt] = xf[ids[e][lo:hi]]
            im[f"xt{s}"] = np.ascontiguousarray(
                xe.T.reshape(KD, P, size).transpose(1, 0, 2)
                .astype(ndt, copy=False))
            wg_t, wu_t, wd_t = get_w(e)
            im[f"wgt{s}"] = wg_t
            im[f"wut{s}"] = wu_t
            im[f"wdt{s}"] = wd_t
        in_maps.append(im)

    res = run_bass_kernel_spmd(nc, in_maps, core_ids=list(range(E)))
    global LAST_RESULTS
    LAST_RESULTS = res

    out = np.zeros((N, D), dtype=np.float32)
    for core in range(E):
        for s, size in enumerate(sizes):
            e, lo, hi = assign[core][s]
            cnt = hi - lo
            if not cnt:
                continue
            y_e = res.results[core][f"yt{s}"].reshape(DIM, size)[:, :cnt].T
            out[ids[e][lo:hi]] += wts[e][lo:hi, None] * y_e
    return out.reshape(B, T, D)


# revision 4
# speedup vs baseline: 1.1172x; 1.0033x over previous
"""Expert-parallel MoE (top-2 of 8 experts, SwiGLU FFN) for 8 Trainium2 cores.

Strategy (expert-parallel per the sharding hint, plus load balancing):
  - Host computes the small gate (logits -> top-2 -> softmax) in float64
    numpy, then dispatches tokens to experts.
  - Device work is the 3-matmul SwiGLU FFN in bf16 (1 PE cycle/row, FWL
    hides LDWEIGHTS; rel err ~4e-3 vs the 2e-2 gate). fp8-DoubleRow was
    measured unusable: e4m3 on any one matmul already gives >3.7e-2.
  - Load balancing: per-expert token counts are imbalanced (max 2151 vs
    mean 2048).  Instead of padding every core to the max count, each
    core runs THREE fixed-size slots (a, b, c); the biggest expert's
    tokens span two a-slots (on different cores), the smallest expert
    donates capacity by using two b-slots, and everyone gets one
    c-slot.  This brings per-core capacity S = a+b+c down to
    max(d2, ceil((d1+d8)/2)) ~ 2068 instead of 2152.  Each slot streams
    its own expert's weights (3 x 17.3MB bf16 per core, well under the
    HBM roofline vs ~455us of compute).
  - The device kernel works in "feature-major" layout (features on
    partitions, tokens on the free axis) so h = silu(x@Wg)*(x@Wu)
    feeds the down-projection without any transpose.
  - Host applies routing weights and scatter-adds per-slot outputs
    back into the full [B,T,D] output.
"""

import numpy as np

DIM = 1024
HID = 2816
E = 8
TOPK = 2
P = 128
KD = DIM // P   # 8 k-subtiles (contraction of x@W)
HT = HID // P   # 22 h-subtiles
DT = DIM // P   # 8 d-subtiles (output features)

CHUNK_MAX = 512         # PSUM bank limit (512 fp32 accumulators)

_KERNEL_CACHE = {}
LAST_RESULTS = None  # BassKernelResults of the most recent run (for profiling)


def _ceil_even(v):
    return ((int(v) + 1) // 2) * 2


def _split_chunks(size):
    """Split a slot into even chunks <= CHUNK_MAX (one PSUM bank each),
    near-equal so every chunk's matmuls stay long enough to hide
    LDWEIGHTS."""
    n = -(-size // CHUNK_MAX)
    sizes = []
    rem = size
    for i in range(n, 0, -1):
        s = rem if i == 1 else min(CHUNK_MAX, ((rem // i + 1) // 2) * 2)
        sizes.append(s)
        rem -= s
    assert rem == 0 and all(0 < s <= CHUNK_MAX and s % 2 == 0 for s in sizes)
    chunks = []
    off = 0
    for s in sizes:
        chunks.append((off, s))
        off += s
    return chunks


def _plan(counts):
    """Choose per-core slot sizes + (expert, lo, hi) piece assignment.

    Slot types (a, b, c), 8 copies of each (one per core):
      biggest expert  -> {a, a, c}   (its surplus spans two cores' a-slots)
      smallest expert -> {b, b, c}   (donates capacity: b < a)
      middle experts  -> {a, b, c}
    Coverage: 2a+c >= d_max, 2b+c >= d_min, a+b+c >= d_2nd, so
    S = a+b+c = max(d_2nd, ceil((d_max+d_min)/2)) is achievable.
    Returns (sizes, assign) with sizes desc-sorted and
    assign[core] = [(expert, lo, hi), ...] parallel to sizes.
    """
    cnts = [int(x) for x in counts]
    order = sorted(range(E), key=lambda e: -cnts[e])
    d = [cnts[e] for e in order]
    d0, d1, dmin = d[0], d[1], d[-1]

    smin = _ceil_even(max(d1, (d0 + dmin + 1) // 2, 6))
    best = None
    for c in range(384, 1153, 32):
        if c >= smin - 4:
            break
        S = smin
        plan_ab = None
        while S <= _ceil_even(d0):
            a = _ceil_even(-(-max(d0 - c, 2) // 2))
            b = S - a - c
            if b >= 2 and 2 * b + c >= dmin:
                plan_ab = (a, b)
                break
            S += 2
        if plan_ab is None:
            continue
        a, b = plan_ab
        minchunk = min(
            min(cs for _, cs in _split_chunks(s)) for s in (a, b, c)
        )
        key = (S, -minchunk)
        if best is None or key < best[0]:
            best = (key, (a, b, c))

    if best is None:
        # fallback: uniform single slot per core (always correct)
        C = _ceil_even(max(max(cnts), 256))
        sizes = [C]
        assign = [[(order[i], 0, cnts[order[i]])] for i in range(E)]
        return sizes, assign

    a, b, c = best[1]

    def pieces(n, caps):
        out = []
        lo = 0
        for cap in caps:
            hi = min(n, lo + cap)
            out.append((lo, hi))
            lo = hi
        assert lo == n
        return out

    a_pieces, b_pieces, c_pieces = [], [], []
    for rank, e in enumerate(order):
        n = cnts[e]
        if rank == 0:
            pa1, pa2, pc = pieces(n, [a, a, c])
            a_pieces += [(e,) + pa1, (e,) + pa2]
            c_pieces.append((e,) + pc)
        elif rank == E - 1:
            pb1, pb2, pc = pieces(n, [b, b, c])
            b_pieces += [(e,) + pb1, (e,) + pb2]
            c_pieces.append((e,) + pc)
        else:
            pa, pb, pc = pieces(n, [a, b, c])
            a_pieces.append((e,) + pa)
            b_pieces.append((e,) + pb)
            c_pieces.append((e,) + pc)
    assert len(a_pieces) == E and len(b_pieces) == E and len(c_pieces) == E

    typed = sorted(
        [(a, a_pieces), (b, b_pieces), (c, c_pieces)], key=lambda t: -t[0]
    )
    sizes = [t[0] for t in typed]
    assign = [[typed[s][1][i] for s in range(len(typed))] for i in range(E)]
    return sizes, assign


def _build_moe_ffn(slot_sizes):
    """Per-core Bass program: for each slot, y^T = SwiGLU FFN of x^T with
    that slot's own expert weights; feature-major, bf16 matmuls."""
    import concourse.bass as bass  # noqa: F401
    import concourse.mybir as mybir
    from concourse import bacc, tile

    f32 = mybir.dt.float32
    dt_in = mybir.dt.bfloat16
    SiLU = mybir.ActivationFunctionType.Silu

    nc = bacc.Bacc("TRN2", target_bir_lowering=False, debug=False)

    xt, wgt, wut, wdt, yt = [], [], [], [], []
    for s, size in enumerate(slot_sizes):
        xt.append(nc.dram_tensor(f"xt{s}", [P, KD, size], dt_in,
                                 kind="ExternalInput"))
        wgt.append(nc.dram_tensor(f"wgt{s}", [HT, P, KD, P], dt_in,
                                  kind="ExternalInput"))
        wut.append(nc.dram_tensor(f"wut{s}", [HT, P, KD, P], dt_in,
                                  kind="ExternalInput"))
        wdt.append(nc.dram_tensor(f"wdt{s}", [DT, P, HT, P], dt_in,
                                  kind="ExternalInput"))
        yt.append(nc.dram_tensor(f"yt{s}", [DT, P, size], f32,
                                 kind="ExternalOutput"))

    with tile.TileContext(nc) as tc:
        with (
            tc.tile_pool(name="xp", bufs=1) as xp,
            tc.tile_pool(name="wp", bufs=3) as wp,
            tc.tile_pool(name="hp", bufs=2) as hp,
            tc.tile_pool(name="op", bufs=3) as op,
            tc.tile_pool(name="ps", bufs=2, space="PSUM") as ps,
        ):
            HH = KD // 2  # wg/wu tiles split in halves for earlier start
            for s, size in enumerate(slot_sizes):
                group = _split_chunks(size)

                # h = silu(x @ Wg) * (x @ Wu), feature-major [HID, size]
                h_sb = hp.tile([P, HT, size], dt_in, tag="h")

                w_cache = {}

                def load_wg(ht, hh, s=s):
                    w1 = wp.tile([P, HH, P], dt_in, tag=f"wg{hh}",
                                 name=f"wg{hh}")
                    nc.sync.dma_start(
                        w1[:], wgt[s][ht, :, hh * HH : (hh + 1) * HH])
                    return w1

                def load_wu(ht, hh, s=s):
                    w2 = wp.tile([P, HH, P], dt_in, tag=f"wu{hh}",
                                 name=f"wu{hh}")
                    nc.sync.dma_start(
                        w2[:], wut[s][ht, :, hh * HH : (hh + 1) * HH])
                    return w2

                def load_w(ht):
                    # both wg halves before wu: the opening matmuls of
                    # each h-tile consume wg only
                    return ([load_wg(ht, 0), load_wg(ht, 1)],
                            [load_wu(ht, 0), load_wu(ht, 1)])

                # Opening order interleaves the first chunk's x k-slices
                # with ht=0's weight tiles so the first matmul waits on
                # ~240KB and each successive k-group lands just in time.
                x_sb = [None] * len(group)

                def load_x(gi, off, csize, s=s):
                    x0 = xp.tile([P, csize], dt_in, tag=f"x{gi}k0",
                                 bufs=2 if gi == 0 else 1, name=f"x{gi}k0")
                    nc.sync.dma_start(x0[:], xt[s][:, 0, off : off + csize])
                    xa = xp.tile([P, HH - 1, csize], dt_in, tag=f"x{gi}a",
                                 bufs=2 if gi == 0 else 1, name=f"x{gi}a")
                    nc.sync.dma_start(xa[:], xt[s][:, 1:HH, off : off + csize])
                    wg0 = load_wg(0, 0) if gi == 0 and s == 0 else None
                    xb = xp.tile([P, KD - HH, csize], dt_in, tag=f"x{gi}b",
                                 bufs=2 if gi == 0 else 1, name=f"x{gi}b")
                    nc.sync.dma_start(xb[:], xt[s][:, HH:, off : off + csize])
                    x_sb[gi] = ([x0] + [xa[:, kt] for kt in range(HH - 1)]
                                + [xb[:, kt] for kt in range(KD - HH)])
                    return wg0

                if s == 0:
                    off0, csize0 = group[0]
                    wg0 = load_x(0, off0, csize0)
                    w_cache[0] = ([wg0, load_wg(0, 1)],
                                  [load_wu(0, 0), load_wu(0, 1)])
                    w_cache[1] = load_w(1)
                    for gi, (off, csize) in enumerate(group):
                        if gi:
                            load_x(gi, off, csize)
                else:
                    w_cache[0] = load_w(0)
                    w_cache[1] = load_w(1)
                    for gi, (off, csize) in enumerate(group):
                        load_x(gi, off, csize)

                for ht in range(HT):
                    if ht not in w_cache:
                        w_cache[ht] = load_w(ht)
                    wg_sb, wu_sb = w_cache.pop(ht)

                    for gi, (off, csize) in enumerate(group):
                        pg = ps.tile([P, csize], f32, tag="pg", bufs=3)
                        pu = ps.tile([P, csize], f32, tag="pu", bufs=3)
                        for kt in range(KD):
                            nc.tensor.matmul(
                                pg,
                                wg_sb[kt // HH][:, kt % HH],
                                x_sb[gi][kt],
                                start=(kt == 0),
                                stop=(kt == KD - 1),
                            )
                        for kt in range(KD):
                            nc.tensor.matmul(
                                pu,
                                wu_sb[kt // HH][:, kt % HH],
                                x_sb[gi][kt],
                                start=(kt == 0),
                                stop=(kt == KD - 1),
                            )
                        sl = op.tile([P, csize], f32, tag="silu")
                        nc.scalar.activation(sl[:], pg, SiLU)
                        nc.vector.tensor_mul(
                            h_sb[:, ht, off : off + csize], sl[:], pu
                        )

                # y = h @ Wd, feature-major [DIM, size]
                for dt in range(DT):
                    wd_sb = wp.tile([P, HT, P], dt_in, tag="wd")
                    nc.sync.dma_start(wd_sb[:], wdt[s][dt])
                    for gi, (off, csize) in enumerate(group):
                        py = ps.tile([P, csize], f32, tag="py")
                        for ht in range(HT):
                            nc.tensor.matmul(
                                py,
                                wd_sb[:, ht],
                                h_sb[:, ht, off : off + csize],
                                start=(ht == 0),
                                stop=(ht == HT - 1),
                            )
                        o_sb = op.tile([P, csize], f32, tag="o")
                        nc.vector.tensor_copy(o_sb[:], py)
                        nc.sync.dma_start(
                            yt[s][dt, :, off : off + csize], o_sb[:])

    nc.finalize()
    return nc


def _get_kernel(slot_sizes):
    key = tuple(slot_sizes)
    if key not in _KERNEL_CACHE:
        _KERNEL_CACHE[key] = _build_moe_ffn(list(slot_sizes))
    return _KERNEL_CACHE[key]


def _np_bf16():
    import ml_dtypes

    return np.dtype(ml_dtypes.bfloat16)


def _route(xf, W_gate):
    """Replicate reference routing: top-2 by logit, softmax weights.

    float64 logits: the top-k decision boundary gap is >> f32 rounding
    noise, so this matches the f32 jax reference's selection."""
    logits = xf.astype(np.float64) @ W_gate.astype(np.float64)  # [N, E]
    order = np.argsort(-logits, axis=1, kind="stable")[:, :TOPK]  # [N, 2]
    top = np.take_along_axis(logits, order, axis=1)
    top = top - top.max(axis=1, keepdims=True)
    ew = np.exp(top)
    w = (ew / ew.sum(axis=1, keepdims=True)).astype(np.float32)  # [N, 2]
    return order, w


def kernel(x, W_gate, Wg, Wu, Wd):
    from concourse.bass_utils import run_bass_kernel_spmd

    x = np.ascontiguousarray(np.asarray(x, dtype=np.float32))
    W_gate = np.asarray(W_gate, dtype=np.float32)
    Wg = np.asarray(Wg, dtype=np.float32)
    Wu = np.asarray(Wu, dtype=np.float32)
    Wd = np.asarray(Wd, dtype=np.float32)

    B, T, D = x.shape
    xf = x.reshape(-1, D)
    N = xf.shape[0]

    order, w = _route(xf, W_gate)

    ids = []  # per-expert token indices
    wts = []  # per-expert combine weights
    for e in range(E):
        sel = np.nonzero(order == e)
        ids.append(sel[0])
        wts.append(w[sel[0], sel[1]])

    sizes, assign = _plan([len(i) for i in ids])
    nc = _get_kernel(sizes)
    ndt = _np_bf16()

    # transpose + bf16-cast each expert's weights once
    wcache = {}

    def get_w(e):
        if e not in wcache:
            wg_t = np.ascontiguousarray(
                Wg[e].reshape(KD, P, HT, P).transpose(2, 1, 0, 3)
                .astype(ndt, copy=False))
            wu_t = np.ascontiguousarray(
                Wu[e].reshape(KD, P, HT, P).transpose(2, 1, 0, 3)
                .astype(ndt, copy=False))
            wd_t = np.ascontiguousarray(
                Wd[e].reshape(HT, P, DT, P).transpose(2, 1, 0, 3)
                .astype(ndt, copy=False))
            wcache[e] = (wg_t, wu_t, wd_t)
        return wcache[e]

    in_maps = []
    for core in range(E):
        im = {}
        for s, size in enumerate(sizes):
            e, lo, hi = assign[core][s]
            cnt = hi - lo
            xe = np.zeros((size, DIM), dtype=np.float32)
            if cnt:
                xe[:cnt] = xf[ids[e][lo:hi]]
            im[f"xt{s}"] = np.ascontiguousarray(
                xe.T.reshape(KD, P, size).transpose(1, 0, 2)
                .astype(ndt, copy=False))
            wg_t, wu_t, wd_t = get_w(e)
            im[f"wgt{s}"] = wg_t
            im[f"wut{s}"] = wu_t
            im[f"wdt{s}"] = wd_t
        in_maps.append(im)

    res = run_bass_kernel_spmd(nc, in_maps, core_ids=list(range(E)))
    global LAST_RESULTS
    LAST_RESULTS = res

    out = np.zeros((N, D), dtype=np.float32)
    for core in range(E):
        for s, size in enumerate(sizes):
            e, lo, hi = assign[core][s]
            cnt = hi - lo
            if not cnt:
                continue
            y_e = res.results[core][f"yt{s}"].reshape(DIM, size)[:, :cnt].T
            out[ids[e][lo:hi]] += wts[e][lo:hi, None] * y_e
    return out.reshape(B, T, D)
